# revision 10
# baseline (speedup 1.0000x reference)
"""Trainium2 Bass kernel for nn_ComprehensiveNormalization.

Strategy (8 NeuronCores, data-parallel over the 8192 tokens, 1024 each):

Host-side algebra (exact, float64):
  - w = softmax(aw); fold w into the 6 blocks of int_W1.
  - m/n/r state paths: (x + M[b]) @ A = x @ A + M[b] @ A, so the three
    x-blocks collapse into one folded matrix Vx and per-batch constant rows.
  - All additive terms (cp/tm/ms betas through their blocks, state-MLP
    constants, int_b1) become 18 extra matmul K-rows fed by a one-hot input.
Device per token (fp32/fp16 LN math, fp16 matmul operands, fp32 PSUM):
  xhat -> y = xhat*gp+bp -> h = (y-m_y)*rs_y*gc ; t = xhat*gt ; s = xhat*gs
  variants live in one [128tok, 4*1024] tile; a single DMA-XBAR transpose
  per token-tile produces actT [128d, 32chunk, 128tok] (no PE transposes).
  u = [h|t|x|s] @ Wc + onehot18 @ Wtbl ; v = silu(u) ; o = v @ W2 (+b2)
  Final LN runs in transposed layout: column stats via ones-matmuls,
  row broadcast via K=1 outer products, out lands as [D, TPC] in DRAM
  and the host transposes it back.
"""

import os
import sys

sys.path.insert(0, "/opt/trn_rl_repo")

import numpy as np

import concourse.bass as bass
import concourse.tile as tile
from concourse import bacc, mybir
from concourse.bass import IndirectOffsetOnAxis
from concourse.bass_utils import run_bass_kernel_spmd

F32 = mybir.dt.float32
F16 = mybir.dt.float16
I32 = mybir.dt.int32

B, S, D = 4, 2048, 1024
NTOK = B * S              # 8192
NCORES = 8
TPC = NTOK // NCORES      # tokens per core: 1024
NTILES = TPC // 128       # 8 token-tiles per core
HALF = TPC // 2           # 512 tokens per half
KC = 32                   # K chunks of the 4096-row folded weight
NOH = 18                  # one-hot rows
EPS = 1e-5

_CACHED_NC = None


def _build_nc():
    """Build the SPMD Bass program (same program on all 8 cores)."""
    nc = bacc.Bacc("TRN2", target_bir_lowering=False, debug=False,
                   num_devices=NCORES)

    # ---- DRAM parameters (per-core views prepared by the host) ----
    x_d = nc.declare_dram_parameter("x", [TPC, D], F16, isOutput=False)
    pw2_d = nc.declare_dram_parameter("pw2", [1000, 2 * D], F16, isOutput=False)
    cts_d = nc.declare_dram_parameter("cts", [75, 3 * D], F16, isOutput=False)
    # per-token gather row indices, packed [partition, tile]
    pid_d = nc.declare_dram_parameter("pid", [128, NTILES], I32, isOutput=False)
    cid_d = nc.declare_dram_parameter("cid", [128, NTILES], I32, isOutput=False)
    oh_d = nc.declare_dram_parameter("oh", [NOH, TPC], F16, isOutput=False)
    wc_d = nc.declare_dram_parameter("wc", [KC * 128, D], F16, isOutput=False)
    wtbl_d = nc.declare_dram_parameter("wtbl", [NOH, D], F16, isOutput=False)
    w2_d = nc.declare_dram_parameter("w2", [D, D], F16, isOutput=False)
    b2_d = nc.declare_dram_parameter("b2", [128, 8], F32, isOutput=False)
    gi_d = nc.declare_dram_parameter("gi", [128, 8], F32, isOutput=False)
    bi_d = nc.declare_dram_parameter("bi", [128, 8], F32, isOutput=False)
    out_d = nc.declare_dram_parameter("out", [D, TPC], F32, isOutput=True)

    with tile.TileContext(nc) as tc:
        _emit(tc, dict(x=x_d, pw2=pw2_d, cts=cts_d, pid=pid_d, cid=cid_d,
                       oh=oh_d, wc=wc_d, wtbl=wtbl_d, w2=w2_d, b2=b2_d,
                       gi=gi_d, bi=bi_d, out=out_d))
    nc.compile()
    return nc


def _emit(tc, d):
    nc = tc.nc
    from contextlib import ExitStack
    ctx = ExitStack()
    with ctx:
        consts = ctx.enter_context(tc.tile_pool(name="consts", bufs=1))
        wpool = ctx.enter_context(tc.tile_pool(name="weights", bufs=1))
        act_pool = ctx.enter_context(tc.tile_pool(name="actT", bufs=1))
        ln16 = ctx.enter_context(tc.tile_pool(name="ln16", bufs=2))
        varp = ctx.enter_context(tc.tile_pool(name="varp", bufs=2))
        small = ctx.enter_context(tc.tile_pool(name="small", bufs=4))
        vpool = ctx.enter_context(tc.tile_pool(name="vpool", bufs=1))
        opool = ctx.enter_context(tc.tile_pool(name="opool", bufs=1))
        fin = ctx.enter_context(tc.tile_pool(name="fin", bufs=2))
        rows = ctx.enter_context(tc.tile_pool(name="rows", bufs=1))
        ps_l1 = ctx.enter_context(tc.tile_pool(name="ps_l1", bufs=2, space="PSUM"))
        ps_l2 = ctx.enter_context(tc.tile_pool(name="ps_l2", bufs=2, space="PSUM"))
        ps_st = ctx.enter_context(tc.tile_pool(name="ps_st", bufs=1, space="PSUM"))
        ps_bc = ctx.enter_context(tc.tile_pool(name="ps_bc", bufs=1, space="PSUM"))

        # ---- small constants ----
        epsT = consts.tile([128, 1], F32)
        nc.vector.memset(epsT, EPS)
        ones_col = consts.tile([128, 1], F16)
        nc.vector.memset(ones_col, 1.0)
        ones_row = consts.tile([1, 128], F16)
        nc.vector.memset(ones_row, 1.0)
        idx = {}
        for nm in ("pid", "cid"):
            t = consts.tile([128, NTILES], I32, tag=f"idx_{nm}", name=f"idx_{nm}")
            nc.sync.dma_start(out=t[:], in_=d[nm][:])
            idx[nm] = t
        ohT = consts.tile([NOH, TPC], F16, tag="ohT")
        nc.sync.dma_start(out=ohT[:], in_=d["oh"][:])
        wtbl_t = consts.tile([NOH, D], F16, tag="wtbl")
        nc.sync.dma_start(out=wtbl_t[:], in_=d["wtbl"][:])
        gi_t = consts.tile([128, 8], F32, tag="gi")
        nc.sync.dma_start(out=gi_t[:], in_=d["gi"][:])
        bi_t = consts.tile([128, 8], F32, tag="bi")
        nc.sync.dma_start(out=bi_t[:], in_=d["bi"][:])
        b2_t = consts.tile([128, 8], F32, tag="b2")
        nc.sync.dma_start(out=b2_t[:], in_=d["b2"][:])

        # actT[:, tt, v*8+c, :] = variant v, d-chunk c, [d%128, token]
        actT = act_pool.tile([128, NTILES, 4 * 8, 128], F16, tag="actT",
                             name="actT")

        def stats(src_ap, tag):
            st = small.tile([128, 2, 6], F32, tag=f"st_{tag}", name=f"st_{tag}")
            nc.vector.bn_stats(out=st[:, 0, :], in_=src_ap[:, 0:512])
            nc.vector.bn_stats(out=st[:, 1, :], in_=src_ap[:, 512:1024])
            mv = small.tile([128, 2], F32, tag=f"mv_{tag}", name=f"mv_{tag}")
            nc.vector.bn_aggr(out=mv[:], in_=st[:])
            rs = small.tile([128, 1], F32, tag=f"rs_{tag}", name=f"rs_{tag}")
            nc.scalar.activation(out=rs[:], in_=mv[:, 1:2],
                                 func=mybir.ActivationFunctionType.Sqrt,
                                 bias=epsT[:], scale=1.0)
            nc.vector.reciprocal(out=rs[:], in_=rs[:])
            return mv[:, 0:1], rs[:]

        def phase_a_tile(tt):
            # variants tile: [tok, (h | t | x | s) * 1024]
            var16 = varp.tile([128, 4, D], F16, tag="var16", name="var16")
            nc.sync.dma_start(out=var16[:, 2, :],
                              in_=d["x"][tt * 128:(tt + 1) * 128, :])
            gpb = ln16.tile([128, 2 * D], F16, tag="gpb")
            nc.gpsimd.indirect_dma_start(
                out=gpb[:], out_offset=None, in_=d["pw2"][:],
                in_offset=IndirectOffsetOnAxis(ap=idx["pid"][:, tt:tt + 1], axis=0))
            ctst = ln16.tile([128, 3 * D], F16, tag="ctst", bufs=1)
            nc.gpsimd.indirect_dma_start(
                out=ctst[:], out_offset=None, in_=d["cts"][:],
                in_offset=IndirectOffsetOnAxis(ap=idx["cid"][:, tt:tt + 1], axis=0))

            x16 = var16[:, 2, :]
            m_x, rs_x = stats(x16, "x")
            nmrs = small.tile([128, 1], F32, tag="nmrs")
            nc.vector.scalar_tensor_tensor(
                out=nmrs[:], in0=m_x, scalar=-1.0, in1=rs_x,
                op0=mybir.AluOpType.mult, op1=mybir.AluOpType.mult)
            xhat = varp.tile([128, D], F16, tag="xhat", name="xhat")
            nc.scalar.activation(out=xhat[:], in_=x16,
                                 func=mybir.ActivationFunctionType.Identity,
                                 bias=nmrs[:], scale=rs_x)

            y_t = varp.tile([128, D], F16, tag="y", name="y", bufs=1)
            nc.vector.tensor_tensor(out=y_t[:], in0=xhat[:], in1=gpb[:, 0:D],
                                    op=mybir.AluOpType.mult)
            nc.vector.tensor_tensor(out=y_t[:], in0=y_t[:], in1=gpb[:, D:2 * D],
                                    op=mybir.AluOpType.add)
            m_y, rs_y = stats(y_t, "y")

            gcr = small.tile([128, D], F16, tag="gcr", bufs=2)
            nc.vector.tensor_scalar_mul(gcr[:], ctst[:, 0:D], rs_y)
            nc.vector.scalar_tensor_tensor(
                out=var16[:, 0, :], in0=y_t[:], scalar=m_y, in1=gcr[:],
                op0=mybir.AluOpType.subtract, op1=mybir.AluOpType.mult)
            nc.vector.tensor_tensor(out=var16[:, 1, :], in0=xhat[:],
                                    in1=ctst[:, D:2 * D],
                                    op=mybir.AluOpType.mult)
            nc.gpsimd.tensor_tensor(out=var16[:, 3, :], in0=xhat[:],
                                    in1=ctst[:, 2 * D:3 * D],
                                    op=mybir.AluOpType.mult)

            # one XBAR transpose for all 4 variants of this token tile
            nc.scalar.dma_start_transpose(out=actT[:, tt, :, :], in_=var16[:])

        def load_wc(g):
            # half-width weight tiles for uc-group g (out-cols g*512..)
            wc_t = []
            for kb in range(4):
                t = wpool.tile([128, 8, HALF], F16, tag=f"wc{kb}",
                               name=f"wc{kb}g{g}")
                nc.sync.dma_start(
                    out=t[:],
                    in_=d["wc"][kb * 1024:(kb + 1) * 1024,
                                g * HALF:(g + 1) * HALF].rearrange(
                        "(j p) d -> p j d", p=128))
                wc_t.append(t)
            return wc_t

        def load_w2():
            w2_t = wpool.tile([128, 8, D], F16, tag="w2", name="w2")
            nc.sync.dma_start(
                out=w2_t[:],
                in_=d["w2"].rearrange("(j p) d -> p j d", p=128))
            return w2_t

        def warm(n):
            # keep the PE p-state up during the LN phase; reuses the
            # broadcast psum slot (nothing reads warm output)
            for _ in range(n):
                wf = ps_bc.tile([128, HALF], F32, tag="prsb", name="wf")
                nc.tensor.matmul(out=wf[:], lhsT=wtbl_t[:, 0:128],
                                 rhs=ohT[:, 0:HALF], start=True, stop=True)

        def phase_l1(half, g, wc_t, v_t):
            for u4 in range(4):
                uc = g * 4 + u4
                pu = ps_l1.tile([128, HALF], F32, tag="pu", name="pu")
                for kc in range(KC):
                    nc.tensor.matmul(
                        out=pu[:],
                        lhsT=wc_t[kc // 8][:, kc % 8, u4 * 128:(u4 + 1) * 128],
                        rhs=actT[:, half * 4:(half + 1) * 4, kc, :],
                        start=(kc == 0), stop=False)
                nc.tensor.matmul(out=pu[:],
                                 lhsT=wtbl_t[:, uc * 128:(uc + 1) * 128],
                                 rhs=ohT[:, half * HALF:(half + 1) * HALF],
                                 start=False, stop=True)
                nc.scalar.activation(out=v_t[uc][:], in_=pu[:],
                                     func=mybir.ActivationFunctionType.Silu)

        def phase_l2(half, v_t, w2_t):
            o16 = opool.tile([128, 8, HALF], F16, tag="o16", name="o16")
            for oc in range(8):
                po = ps_l2.tile([128, HALF], F32, tag="po", name="po")
                for uc in range(8):
                    nc.tensor.matmul(out=po[:],
                                     lhsT=w2_t[:, uc, oc * 128:(oc + 1) * 128],
                                     rhs=v_t[uc][:],
                                     start=(uc == 0), stop=(uc == 7))
                nc.scalar.activation(out=o16[:, oc, :], in_=po[:],
                                     func=mybir.ActivationFunctionType.Identity,
                                     bias=b2_t[:, oc:oc + 1], scale=1.0)
            return o16

        def final_stats(half, o16):
            # stats over features (partitions): ones-matmul column sums
            pso = ps_st.tile([1, HALF], F32, tag="pso", name="pso")
            psq = ps_st.tile([1, HALF], F32, tag="psq", name="psq")
            for oc in range(8):
                nc.tensor.matmul(out=pso[:], lhsT=ones_col[:],
                                 rhs=o16[:, oc, :],
                                 start=(oc == 0), stop=(oc == 7))
            for oc in range(8):
                osq = fin.tile([128, HALF], F16, tag="osq", name="osq", bufs=2)
                nc.vector.tensor_tensor(out=osq[:], in0=o16[:, oc, :],
                                        in1=o16[:, oc, :],
                                        op=mybir.AluOpType.mult)
                nc.tensor.matmul(out=psq[:], lhsT=ones_col[:],
                                 rhs=osq[:],
                                 start=(oc == 0), stop=(oc == 7))
            m_row = rows.tile([1, HALF], F32, tag="m_row", name="m_row")
            nc.vector.tensor_scalar_mul(m_row[:], pso[:], 1.0 / D)
            msq = rows.tile([1, HALF], F32, tag="msq", name="msq")
            nc.vector.tensor_tensor(out=msq[:], in0=m_row[:], in1=m_row[:],
                                    op=mybir.AluOpType.mult)
            var_row = rows.tile([1, HALF], F32, tag="var_row", name="var_row")
            nc.vector.scalar_tensor_tensor(
                out=var_row[:], in0=psq[:], scalar=1.0 / D, in1=msq[:],
                op0=mybir.AluOpType.mult, op1=mybir.AluOpType.subtract)
            sd = rows.tile([1, HALF], F32, tag="sd", name="sd")
            nc.scalar.activation(out=sd[:], in_=var_row[:],
                                 func=mybir.ActivationFunctionType.Sqrt,
                                 bias=epsT[0:1, :], scale=1.0)
            nc.vector.reciprocal(out=sd[:], in_=sd[:])
            rs16 = rows.tile([1, HALF], F16, tag="rs16", name="rs16")
            nc.vector.tensor_copy(out=rs16[:], in_=sd[:])
            mrs16 = rows.tile([1, HALF], F16, tag="mrs16", name="mrs16")
            nc.vector.tensor_tensor(out=mrs16[:], in0=m_row[:], in1=sd[:],
                                    op=mybir.AluOpType.mult)
            return rs16, mrs16

        def final_affine(half, o16, rs16, mrs16):
            prsb = ps_bc.tile([128, HALF], F32, tag="prsb", name="prsb")
            nc.tensor.matmul(out=prsb[:], lhsT=ones_row[:], rhs=rs16[:],
                             start=True, stop=True)
            pmrsb = ps_bc.tile([128, HALF], F32, tag="pmrsb", name="pmrsb")
            nc.tensor.matmul(out=pmrsb[:], lhsT=ones_row[:], rhs=mrs16[:],
                             start=True, stop=True)
            for oc in range(8):
                z = fin.tile([128, HALF], F16, tag="z", bufs=2)
                nc.vector.tensor_tensor(out=z[:], in0=o16[:, oc, :],
                                        in1=prsb[:], op=mybir.AluOpType.mult)
                nc.vector.tensor_tensor(out=z[:], in0=z[:], in1=pmrsb[:],
                                        op=mybir.AluOpType.subtract)
                outc = fin.tile([128, HALF], F32, tag="outc", bufs=2)
                nc.scalar.activation(out=outc[:], in_=z[:],
                                     func=mybir.ActivationFunctionType.Identity,
                                     bias=bi_t[:, oc:oc + 1],
                                     scale=gi_t[:, oc:oc + 1])
                nc.sync.dma_start(
                    out=d["out"][oc * 128:(oc + 1) * 128,
                                 half * HALF:(half + 1) * HALF],
                    in_=outc[:])

        # ---- schedule ----
        for tt in range(4):
            phase_a_tile(tt)
        wcg = load_wc(0)
        w2_t = load_w2()
        warm(4)
        for tt in range(4, 8):
            phase_a_tile(tt)
        warm(4)
        v0 = [vpool.tile([128, HALF], F16, tag=f"v0{uc}", name=f"v0{uc}")
              for uc in range(8)]
        v1 = [vpool.tile([128, HALF], F16, tag=f"v1{uc}", name=f"v1{uc}")
              for uc in range(8)]
        phase_l1(0, 0, wcg, v0)
        phase_l1(1, 0, wcg, v1)
        wcg = load_wc(1)
        phase_l1(0, 1, wcg, v0)
        o0 = phase_l2(0, v0, w2_t)
        st0 = final_stats(0, o0)
        phase_l1(1, 1, wcg, v1)
        final_affine(0, o0, *st0)
        o1 = phase_l2(1, v1, w2_t)
        st1 = final_stats(1, o1)
        final_affine(1, o1, *st1)


# ---------------------------------------------------------------------------
# Host-side preparation
# ---------------------------------------------------------------------------

def _ln64(x, g, b):
    m = x.mean(-1, keepdims=True)
    v = ((x - m) ** 2).mean(-1, keepdims=True)
    return (x - m) / np.sqrt(v + EPS) * g + b


def _mlp_ln64(s, W1, b1, W2, b2, g, b):
    h = s @ W1 + b1
    h = h / (1.0 + np.exp(-h))
    h = h @ W2 + b2
    return _ln64(h, g, b)


def _prepare(inp):
    f64 = np.float64
    g = lambda k: np.asarray(inp[k], f64)
    aw = g("aw")
    w = np.exp(aw - aw.max())
    w = w / w.sum()
    W1 = g("int_W1")
    A = [W1[i * D:(i + 1) * D] for i in range(6)]
    V0, V1, V5 = w[0] * A[0], w[1] * A[1], w[5] * A[5]
    Vx = w[2] * A[2] + w[3] * A[3] + w[4] * A[4]
    Wc = np.concatenate([V0, V1, Vx, V5], 0)

    M = _mlp_ln64(g("memory_state"), g("mem_W1"), g("mem_b1"), g("mem_W2"),
                  g("mem_b2"), g("mem_g"), g("mem_be"))
    N = _mlp_ln64(g("noise_state"), g("noi_W1"), g("noi_b1"), g("noi_W2"),
                  g("noi_b2"), g("noi_g"), g("noi_be"))
    R = _mlp_ln64(g("resource_state"), g("res_W1"), g("res_b1"), g("res_W2"),
                  g("res_b2"), g("res_g"), g("res_be"))
    c_b = M @ (w[2] * A[2]) + N @ (w[3] * A[3]) + R @ (w[4] * A[4])

    Wtbl = np.zeros((NOH, D), f64)
    Wtbl[0:5] = g("cp_b") @ V0
    Wtbl[5:10] = g("tm_b") @ V1
    Wtbl[10:13] = g("ms_b") @ V5
    Wtbl[13:17] = c_b
    Wtbl[17] = g("int_b1")

    pid = np.asarray(inp["pathway_ids"]).reshape(-1).astype(np.int32)
    cid = np.asarray(inp["compartment_ids"]).reshape(-1).astype(np.int32)
    tid = np.asarray(inp["time_steps"]).reshape(-1).astype(np.int32)
    sid = np.asarray(inp["scale_type"]).reshape(-1).astype(np.int32)
    bix = np.repeat(np.arange(B, dtype=np.int32), S)
    ctsid = cid * 15 + tid * 3 + sid

    oh = np.zeros((NTOK, NOH), np.float16)
    ar = np.arange(NTOK)
    oh[ar, cid] = 1
    oh[ar, 5 + tid] = 1
    oh[ar, 10 + sid] = 1
    oh[ar, 13 + bix] = 1
    oh[:, 17] = 1

    # combined gather tables
    pw2 = np.concatenate([np.asarray(inp["pw_g"], np.float32),
                          np.asarray(inp["pw_b"], np.float32)], 1)
    cg = np.asarray(inp["cp_g"], np.float32)
    tg = np.asarray(inp["tm_g"], np.float32)
    sg = np.asarray(inp["ms_g"], np.float32)
    cts = np.zeros((75, 3 * D), np.float32)
    for c in range(5):
        for t in range(5):
            for s_ in range(3):
                r = c * 15 + t * 3 + s_
                cts[r, 0:D] = cg[c]
                cts[r, D:2 * D] = tg[t]
                cts[r, 2 * D:3 * D] = sg[s_]

    x = np.ascontiguousarray(
        np.asarray(inp["x"], np.float32).reshape(NTOK, D)).astype(np.float16)
    shared = {
        "pw2": pw2.astype(np.float16),
        "cts": cts.astype(np.float16),
        "wc": Wc.astype(np.float16),
        "wtbl": Wtbl.astype(np.float16),
        "w2": np.asarray(inp["int_W2"], np.float32).astype(np.float16),
        "b2": np.ascontiguousarray(
            np.asarray(inp["int_b2"], np.float32).reshape(8, 128).T),
        "gi": np.ascontiguousarray(
            np.asarray(inp["int_g"], np.float32).reshape(8, 128).T),
        "bi": np.ascontiguousarray(
            np.asarray(inp["int_be"], np.float32).reshape(8, 128).T),
    }

    def pack_idx(a, c):
        return np.ascontiguousarray(
            a[c * TPC:(c + 1) * TPC].reshape(NTILES, 128).T)

    in_maps = []
    for c in range(NCORES):
        m = dict(shared)
        m["x"] = x[c * TPC:(c + 1) * TPC]
        m["pid"] = pack_idx(pid, c)
        m["cid"] = pack_idx(ctsid, c)
        m["oh"] = np.ascontiguousarray(oh[c * TPC:(c + 1) * TPC].T)
        in_maps.append(m)
    return in_maps


def kernel(**inputs):
    global _CACHED_NC
    if _CACHED_NC is None:
        _CACHED_NC = _build_nc()
    nc = _CACHED_NC
    in_maps = _prepare(inputs)
    res = run_bass_kernel_spmd(nc, in_maps, list(range(NCORES)),
                               trace=bool(os.environ.get("BASS_TRACE")))
    kernel._last = res
    out = np.concatenate([res.results[c]["out"].T for c in range(NCORES)], 0)
    return out.reshape(B, S, D).astype(np.float32)
